# revision 14
# baseline (speedup 1.0000x reference)
"""Masked multi-head attention block (B=8, N=1024, D=768, H=12) on 8 NeuronCores.

Strategy: pure data-parallel over batch (1 batch element per core).  Per core,
the whole attention block runs in a transpose-free dataflow:

  phase 1a: qkT[e, n]  = WqkvT(lhsT) @ xT        (q,k in [head_dim, seq] layout)
  phase 1b: v[n, e]    = xT(lhsT) @ WvT          (v in natural [seq, head_dim] layout,
                                                  stored interleaved with a ones column)
  phase 2:  ST[j, i]   = kT(lhsT, K=64) @ qT     (scores TRANSPOSED: softmax axis on
                                                  partitions; head pairs run concurrently
                                                  in the two 64-row halves of the PE array)
            P = exp(ST*scale + key_mask_bias)    (ACT, per-partition bias kills masked keys)
            P[i,i] += (1-m_i)*1e15               (diag add; makes padded-query columns
                                                  one-hot after normalization, to fp32
                                                  precision, since G dominates the sums)
  phase 3:  OT'[d+1, i] = Vaug(lhsT) @ P         (ones column of Vaug yields the softmax
                                                  denominator Z as row 64 for free)
            R = 1/Z (recip_approx_fast), Rb = ones x R  (PE K=1 broadcast matmul, fp32)
            otn = OT'[0:64] * Rb                 (normalized attn output, transposed)
  phase 4:  out[n, e]  = otn(lhsT) @ WprojT + ones(K=1) x bproj

All big matmuls run in float32r (full PE rate at moving-dim >= 256; ~2e-4 relative).
Input DMAs are consolidated and spread across the three DMA-capable queues
(sync / scalar / gpsimd) so descriptor generation does not serialize the prologue.
"""
import sys
for _p in ('/opt/trn_rl_repo',):
    if _p not in sys.path:
        sys.path.insert(0, _p)

from contextlib import ExitStack

import numpy as np

import concourse.bass as bass
import concourse.bacc as bacc
import concourse.mybir as mybir
import concourse.tile as tile
from concourse import bass_utils

F32 = mybir.dt.float32
F32R = mybir.dt.float32r
AF = mybir.ActivationFunctionType

B, N, D, H, HD = 8, 1024, 768, 12, 64
P = 128
DT = D // P            # 6 d-tiles
SCALE = HD ** -0.5
NEGMASK = -30000.0     # exp(x + NEGMASK) == 0.0 in fp32 for any realistic score
BIGG = 1e15            # diagonal dominance constant for padded-query rows


def build_nc(n=N, debug=False):
    NT = n // P                    # seq tiles (8)
    CH = min(512, n)               # matmul moving-dim chunk
    NCH = n // CH                  # chunks (2)

    nc = bacc.Bacc("TRN2", target_bir_lowering=False, debug=False)

    xT_d = nc.dram_tensor("xT", [D, n], F32, kind="ExternalInput")
    wqkvT_d = nc.dram_tensor("wqkvT", [D, 3 * D], F32, kind="ExternalInput")
    wprojT_d = nc.dram_tensor("wprojT", [D, D], F32, kind="ExternalInput")
    bproj_d = nc.dram_tensor("bproj", [1, D], F32, kind="ExternalInput")
    mbias_d = nc.dram_tensor("mbias", [P, NT], F32, kind="ExternalInput")
    omm_d = nc.dram_tensor("omm", [P, NT], F32, kind="ExternalInput")
    ones_d = nc.dram_tensor("onesv", [1, P], F32, kind="ExternalInput")
    out_d = nc.dram_tensor("out", [n, D], F32, kind="ExternalOutput")

    def rr(ap):
        return ap.bitcast(F32R)

    with tile.TileContext(nc) as tc, ExitStack() as ctx:
        persist = ctx.enter_context(tc.tile_pool(name="persist", bufs=1))
        qk = persist.tile([P, 2 * DT, n], F32R)       # e-tiles: 0..5 = q, 6..11 = k
        vaug = persist.tile([P, NT, H, HD + 1], F32R)  # v natural + ones column
        otn = persist.tile([P, DT, n], F32R)          # normalized attn out, transposed
        dtl = persist.tile([P, NT, P], F32R)          # diag((1-m)*G) blocks
        mb = persist.tile([P, NT], F32)
        om = persist.tile([P, NT], F32R)
        ones = persist.tile([1, P], F32R)
        ones_f = persist.tile([1, P], F32)
        bpj = persist.tile([1, D], F32R)

        nc.sync.dma_start(mb, mbias_d.ap())
        nc.sync.dma_start(om, rr(omm_d.ap()))
        nc.sync.dma_start(ones, rr(ones_d.ap()))
        nc.sync.dma_start(ones_f, ones_d.ap())
        nc.sync.dma_start(bpj, rr(bproj_d.ap()))
        # ones column of vaug via broadcast DMA (memset cannot write f32r)
        ones_bc = bass.AP(tensor=rr(ones_d.ap()).tensor, offset=0,
                          ap=[[0, P], [0, NT * H]])
        nc.gpsimd.dma_start(vaug[:, :, :, HD].rearrange("p a b -> p (a b)"), ones_bc)
        for t in range(NT):
            nc.gpsimd.affine_select(
                out=dtl[:, t, :],
                in_=om[:, t:t + 1].to_broadcast((P, P)),
                pattern=[[-1, P]],
                compare_op=mybir.AluOpType.is_equal,
                fill=0.0, base=0, channel_multiplier=1,
            )

        engs = [nc.sync, nc.scalar, nc.gpsimd]

        # ---------------- phase 1: projections ----------------
        with tc.tile_pool(name="ph1x", bufs=1) as ph1x:
            xt = ph1x.tile([P, DT, n], F32R)
            xt_src = rr(xT_d.ap()).rearrange("(dt p) n -> p dt n", p=P)
            for d in range(DT):
                for q2 in range(2):
                    w2 = n // 2
                    engs[(2 * d + q2) % 3].dma_start(
                        xt[:, d, q2 * w2:(q2 + 1) * w2],
                        xt_src[:, d, q2 * w2:(q2 + 1) * w2])

            # 1a: q,k transposed  (qkT[e-tile, :] = sum_d WqkvT[d, e].T @ xT[d, :])
            with tc.tile_pool(name="ph1wa", bufs=1) as ph1wa, \
                 tc.tile_pool(name="pp1", bufs=3, space="PSUM") as pp1:
                wqa = ph1wa.tile([P, DT, 2 * D], F32R)
                wq_src = rr(wqkvT_d.ap()).rearrange("(dt p) e -> p dt e", p=P)
                for d in range(DT):
                    for q2 in range(2):
                        w2 = D
                        engs[(2 * d + q2 + 1) % 3].dma_start(
                            wqa[:, d, q2 * w2:(q2 + 1) * w2],
                            wq_src[:, d, q2 * w2:(q2 + 1) * w2])
                for E in range(2 * DT):
                    for c in range(NCH):
                        ps = pp1.tile([P, CH], F32, tag="pp1")
                        for d in range(DT):
                            nc.tensor.matmul(ps, wqa[:, d, E * P:(E + 1) * P],
                                             xt[:, d, c * CH:(c + 1) * CH],
                                             start=(d == 0), stop=(d == DT - 1))
                        nc.vector.tensor_copy(qk[:, E, c * CH:(c + 1) * CH], ps)

            # 1b: v natural, scattered into vaug's per-head 65-wide blocks
            with tc.tile_pool(name="ph1wb", bufs=1) as ph1wb, \
                 tc.tile_pool(name="pp2", bufs=3, space="PSUM") as pp2:
                wqb = ph1wb.tile([P, DT, D], F32R)
                for d in range(DT):
                    engs[d % 3].dma_start(wqb[:, d, :], wq_src[:, d, 2 * D:3 * D])
                for t in range(NT):
                    for (cb, cw) in ((0, 512), (512, 256)):
                        psf = pp2.tile([P, 512], F32, tag="pp2", name="pp2")
                        ps = psf[:, :cw]
                        for d in range(DT):
                            nc.tensor.matmul(ps, xt[:, d, t * P:(t + 1) * P],
                                             wqb[:, d, cb:cb + cw],
                                             start=(d == 0), stop=(d == DT - 1))
                        h0 = cb // HD
                        nc.vector.tensor_copy(
                            vaug[:, t, h0:h0 + cw // HD, 0:HD],
                            ps.rearrange("p (h d) -> p h d", d=HD))

        # ---------------- phases 2+3: attention ----------------
        with tc.tile_pool(name="pP", bufs=1) as pP, \
             tc.tile_pool(name="znorm", bufs=2) as znorm, \
             tc.tile_pool(name="stps", bufs=2, space="PSUM") as stps, \
             tc.tile_pool(name="otps", bufs=2, space="PSUM") as otps, \
             tc.tile_pool(name="rbps", bufs=2, space="PSUM") as rbps:
            for pr in range(DT):                      # 6 head pairs
                pa = pP.tile([P, NT, n], F32R, tag="pa")
                pb = pP.tile([P, NT, n], F32R, tag="pb")
                pboth = (pa, pb)
                # scores + exp (heads 2pr / 2pr+1 in array halves, concurrent)
                for t in range(NT):
                    for hi, lo in ((0, 0), (1, 64)):
                        st = stps.tile([P, n], F32, tag="st")
                        for c in range(NCH):
                            nc.tensor.matmul(
                                st[:, c * CH:(c + 1) * CH],
                                qk[lo:lo + HD, DT + pr, t * P:(t + 1) * P],
                                qk[lo:lo + HD, pr, c * CH:(c + 1) * CH],
                                start=True, stop=True)
                        nc.scalar.activation(pboth[hi][:, t, :], st, AF.Exp,
                                             bias=mb[:, t:t + 1], scale=SCALE)
                # diagonal dominance add for padded-query columns
                for t in range(NT):
                    for hi in range(2):
                        nc.vector.tensor_add(pboth[hi][:, t, t * P:(t + 1) * P],
                                             pboth[hi][:, t, t * P:(t + 1) * P],
                                             dtl[:, t, :])
                # attn @ v (+ Z row), normalize
                for hi in range(2):
                    h = 2 * pr + hi
                    for c in range(NCH):
                        ot = otps.tile([HD + 1, CH], F32, tag="ot")
                        for t in range(NT):
                            nc.tensor.matmul(ot, vaug[:, t, h, :],
                                             pboth[hi][:, t, c * CH:(c + 1) * CH],
                                             start=(t == 0), stop=(t == NT - 1))
                        z65 = znorm.tile([HD + 1, CH], F32, tag="z65")
                        nc.vector.tensor_copy(z65[HD:HD + 1, :], ot[HD:HD + 1, :])
                        zf = znorm.tile([1, 2, CH], F32, tag="zf")
                        nc.sync.dma_start(zf[:, 0, :], z65[HD:HD + 1, :])  # shift to base 0
                        nc.vector.reciprocal_approx_fast(zf[:, 1, :], zf[:, 0, :])
                        rb = rbps.tile([HD, CH], F32, tag="rb")
                        nc.tensor.matmul(rb, ones_f[:, 0:HD], zf[:, 1, :],
                                         start=True, stop=True)
                        rbs = znorm.tile([HD, CH], F32, tag="rbs")
                        nc.vector.tensor_copy(rbs, rb)
                        if hi == 0:
                            nc.vector.tensor_mul(otn[0:HD, pr, c * CH:(c + 1) * CH],
                                                 ot[0:HD, :], rbs)
                        else:
                            tmp = znorm.tile([HD, CH], F32R, tag="tmp")
                            nc.vector.tensor_mul(tmp, ot[0:HD, :], rbs)
                            nc.sync.dma_start(otn[HD:P, pr, c * CH:(c + 1) * CH], tmp)

        # ---------------- phase 4: output projection ----------------
        with tc.tile_pool(name="ph4w", bufs=1) as ph4w, \
             tc.tile_pool(name="ob", bufs=3) as obp, \
             tc.tile_pool(name="p4", bufs=3, space="PSUM") as p4p:
            wpj = ph4w.tile([P, DT, D], F32R)
            wpj_src = rr(wprojT_d.ap()).rearrange("(dt p) e -> p dt e", p=P)
            for d in range(DT):
                engs[d % 3].dma_start(wpj[:, d, :], wpj_src[:, d, :])
            for t in range(NT):
                ob = obp.tile([P, D], F32, tag="ob")
                for (cb, cw) in ((0, 512), (512, 256)):
                    psf = p4p.tile([P, 512], F32, tag="p4", name="p4")
                    ps = psf[:, :cw]
                    for d in range(DT):
                        nc.tensor.matmul(ps, otn[:, d, t * P:(t + 1) * P],
                                         wpj[:, d, cb:cb + cw],
                                         start=(d == 0), stop=False)
                    nc.tensor.matmul(ps, ones, bpj[:, cb:cb + cw],
                                     start=False, stop=True)
                    nc.vector.tensor_copy(ob[:, cb:cb + cw], ps)
                nc.sync.dma_start(out_d.ap()[t * P:(t + 1) * P, :], ob)

    nc.compile()
    return nc


def make_in_maps(x, mask, Wqkv, Wproj, bproj):
    x = np.ascontiguousarray(np.asarray(x), dtype=np.float32)
    mask = np.asarray(mask)
    wqkvT = np.ascontiguousarray(np.asarray(Wqkv, dtype=np.float32).T)
    wprojT = np.ascontiguousarray(np.asarray(Wproj, dtype=np.float32).T)
    bp = np.ascontiguousarray(np.asarray(bproj, dtype=np.float32).reshape(1, D))
    onesv = np.ones((1, P), dtype=np.float32)
    b, n, _ = x.shape
    nt = n // P
    in_maps = []
    for i in range(b):
        mf = mask[i].astype(np.float32)
        mcol = mf.reshape(nt, P).T.copy()              # [P, NT]
        in_maps.append({
            "xT": np.ascontiguousarray(x[i].T),
            "wqkvT": wqkvT,
            "wprojT": wprojT,
            "bproj": bp,
            "mbias": np.ascontiguousarray((mcol - 1.0) * (-NEGMASK)),
            "omm": np.ascontiguousarray((1.0 - mcol) * BIGG),
            "onesv": onesv,
        })
    return in_maps


_NC_CACHE = {}


def get_nc(n=N):
    if n not in _NC_CACHE:
        _NC_CACHE[n] = build_nc(n)
    return _NC_CACHE[n]


def kernel(x, mask, Wqkv, Wproj, bproj):
    x = np.asarray(x)
    b, n, _ = x.shape
    nc = get_nc(n)
    in_maps = make_in_maps(x, mask, Wqkv, Wproj, bproj)
    res = bass_utils.run_bass_kernel_spmd(nc, in_maps, core_ids=list(range(b)))
    out = np.stack([res.results[i]["out"] for i in range(b)], axis=0)
    return out.astype(np.float32)
